# revision 35
# baseline (speedup 1.0000x reference)
"""Trainium2 Bass kernel for nn_DescrptSeT (DeepMD three-body descriptor).

Self-contained: hardcodes shapes from the problem spec.
  nlist (1,256,96) i32, extended_coord (1,1536) f32, extended_atype (1,512) i32,
  mean/stddev (2,96,4) f32, W1 (3,1,24), b1 (3,24), W2 (3,24,48), b2 (3,48),
  W3 (3,48,96), b3 (3,96) -> out (1,256,96) f32.

Strategy (8 cores, data-parallel over the 256 local atoms, 32 per core):
  stage 1: gather neighbor coords (indirect DMA, one per neighbor slot —
           multi-offset gathers are broken on HW), build rr = diff*sw/l^2 in
           a (128 x 24) wide layout, apply (rr-mean)/std, pair matmuls
           env_p = rr_i^T rr_j on PE. All matmul operands bf16 (4x PE rate
           vs fp32).
  stage 2: flatten env into a 4-lane MLP batch (feat-major) + a batch-major
           one-hot layout for the contraction. MLP 1->24->48->96 with tanh
           and resnet duplication residuals:
             L1 as K=4 blockdiag matmul over 4 lanes, tanh on ACT,
             L2 as 2 packed 2-lane K=64 matmuls into the t2 layout
                [e-f0:24 @0:24 | e-f24:48 @32:56 | ones @56 |
                 o-f0:24 @64:88 | o-f24:48 @96:120 | ones @120]
                (ones rows made by tanh(0+30)=1), t2 = pure tanh(p2),
             L3 flipped, two PSUM-accumulating matmuls per 128-batch chunk:
                lhsT = t2 half (64 x 128) with rhs = [W3;b3 | I24 I24] plus
                lhsT = t1 lane rows (24 x 128) with rhs = [W3sum | I24 I24],
                which folds the h1 resnet residual into both p3 and the h2^T
                passthrough with no DVE adds or SBUF-SBUF DMAs (DVE tensor
                ops require identical operand partitions on HW),
             contraction on PE: lhsT = one-hot env columns (128 x 8), PSUM-
                accumulated per window w = 3q+P (the two atoms' P-segments
                of a q-row), software-pipelined with the next L1/L3 emitted
                before each contraction so PE overlaps the activations.
  stage 3: selection-matrix matmul folds windows*scales -> (32 x 144), final
           DVE add folds the [rh2;rh2] duplication, DMA out.
"""

import sys

sys.path.insert(0, "/opt/trn_rl_repo")

import numpy as np

import concourse.bass as bass
import concourse.tile as tile
from concourse import bacc, mybir
from concourse.bass_utils import run_bass_kernel_spmd

F32 = mybir.dt.float32
BF16 = mybir.dt.bfloat16
I32 = mybir.dt.int32
try:
    NPBF = np.dtype("bfloat16")
except TypeError:
    import ml_dtypes
    NPBF = np.dtype(ml_dtypes.bfloat16)

# problem constants
NCORES = 8
NLOC, NALL, NNEI, NG = 256, 512, 96, 96
SEL = [32, 64]
PAIRS = [(0, 0), (0, 1), (1, 1)]
PAIR_SC = [1.0 / (SEL[ti] * SEL[tj]) for ti, tj in PAIRS]
RCUT, RCUT_SMTH = 6.0, 0.5

A_CORE = NLOC // NCORES            # 32 atoms per core
PER_ATOM = 32 * 32 + 32 * 64 + 64 * 64   # 7168 env elems per atom
B_CORE = A_CORE * PER_ATOM         # 229376
LANES = 4
LANE_COLS = B_CORE // LANES        # 57344 (8 atoms per lane)
NSUPER = LANE_COLS // 512          # 112
CC_LANE = LANE_COLS // 128         # 448 chunks per lane
# env_mlp row layout (row = 32q+l, the q-th quarter of lane l, holding the
# two atoms a0 = 8l+2q, a1 = 8l+2q+1): segments aa-paired and contiguous:
#   [s0a0 0:1024 | s0a1 1024:2048 | s1a0 2048:4096 | s1a1 4096:6144 |
#    s2a0 6144:10240 | s2a1 10240:14336]
# pair index per supertile position within a q-row's 28 supertiles
PAIR_OF_POS = [0] * 4 + [1] * 8 + [2] * 16
SEG_NCH = [8, 16, 32]              # 128-chunks per (atom, pair) segment
SEG_CSTART = [0, 16, 48]           # chunk start of pair seg within a q-row
SEG_CEND = [16, 48, 112]           # chunk end (both atoms)

# windows: w = 3q + P accumulates the two atoms' P-segments of q-row; holds
# 8 acc sections r = 4*aa + lane. For window w: first lane-chunk W0[w],
# length NCHW[w] chunks.
W0 = [112 * (w // 3) + SEG_CSTART[w % 3] for w in range(12)]
NCHW = [2 * SEG_NCH[w % 3] for w in range(12)]
WBASE = [8 * sum(NCHW[:w]) for w in range(12)]        # env_bm col base of window


def _pair_of_ccr(ccr):  # ccr = cc % 112 (chunk within q-row)
    if ccr < 16:
        return 0
    if ccr < 48:
        return 1
    return 2


# ---------------------------------------------------------------- host aux

def _build_static_aux(W1, b1, W2, b2, W3, b3, mean, stddev):
    """Input-weight-derived aux tensors (replicated to all cores)."""
    aux = {}
    # W1rep (3, 128, 128): quarter-q rows 32q+g hold W1 blockdiag row g
    w1blk = np.zeros((3, 4, 128), np.float32)
    for p in range(3):
        for g in range(4):
            w1blk[p, g, 32 * g:32 * g + 24] = W1[p, 0, :]
    w1rep = np.zeros((3, 128, 128), np.float32)
    for q in range(4):
        w1rep[:, 32 * q:32 * q + 4, :] = w1blk
    aux["w1rep"] = w1rep.astype(NPBF)
    # b1t (3, 128, 1)
    b1t = np.zeros((3, 128, 1), np.float32)
    for g in range(4):
        b1t[:, 32 * g:32 * g + 24, 0] = b1
    aux["b1t"] = b1t
    # W2 packed 2-lane blockdiag (3, 128, 121), duplicated at row bases 0/64
    # so lhsT slice [64t:64t+64] works for both t. t2 row layout per lane
    # half (32-aligned so the DVE residual adds have legal start partitions):
    #   even lane: f0:24 @ 0:24, f24:48 @ 32:56, ones @ 56
    #   odd  lane: f0:24 @ 64:88, f24:48 @ 96:120, ones @ 120
    # K rows 0:24 = even-lane t1, K rows 32:56 = odd-lane t1.
    w2pk = np.zeros((3, 128, 128), np.float32)
    for rb in (0, 64):
        w2pk[:, rb + 0:rb + 24, 0:24] = W2[:, :, 0:24]
        w2pk[:, rb + 0:rb + 24, 32:56] = W2[:, :, 24:48]
        w2pk[:, rb + 32:rb + 56, 64:88] = W2[:, :, 0:24]
        w2pk[:, rb + 32:rb + 56, 96:120] = W2[:, :, 24:48]
    aux["w2pk"] = w2pk.astype(NPBF)
    # b2 bias (3, 128, 1) matching the t2 row layout; 30 -> tanh = 1.0
    b2sp = np.zeros((3, 128, 1), np.float32)
    b2sp[:, 0:24, 0] = b2[:, 0:24]
    b2sp[:, 32:56, 0] = b2[:, 24:48]
    b2sp[:, 56, 0] = 30.0
    b2sp[:, 64:88, 0] = b2[:, 0:24]
    b2sp[:, 96:120, 0] = b2[:, 24:48]
    b2sp[:, 120, 0] = 30.0
    aux["b2sp"] = b2sp
    # W3cat (3, 128, 144): per half rb: rows rb:rb+24 = [W3[0:24] | I24@96:120],
    # rows rb+32:rb+56 = [W3[24:48] | I24@120:144], row rb+56 = [b3 | 0]
    w3cat = np.zeros((3, 128, 144), np.float32)
    for rb in (0, 64):
        w3cat[:, rb:rb + 24, 0:96] = W3[:, 0:24]
        w3cat[:, rb + 32:rb + 56, 0:96] = W3[:, 24:48]
        w3cat[:, rb + 56, 0:96] = b3
        w3cat[:, rb:rb + 24, 96:120] = np.eye(24)[None]
        w3cat[:, rb + 32:rb + 56, 120:144] = np.eye(24)[None]
    aux["w3cat"] = w3cat.astype(NPBF)
    # W3t1 (3, 128, 144): second PSUM-accum matmul folds the h1 resnet
    # residual into p3 and h2^T: p3 += W3sum^T h1, h2^T += [h1 | h1].
    # W3sum = W3[0:24] + W3[24:48]; replicated at every 32-aligned row base
    # so lhsT = t1[32l:32l+24] pairs with rhs = w3t1[32l:32l+24].
    w3sum = W3[:, 0:24] + W3[:, 24:48]
    w3t1 = np.zeros((3, 128, 144), np.float32)
    for rb in (0, 32, 64, 96):
        w3t1[:, rb:rb + 24, 0:96] = w3sum
        w3t1[:, rb:rb + 24, 96:120] = np.eye(24)[None]
        w3t1[:, rb:rb + 24, 120:144] = np.eye(24)[None]
    aux["w3t1"] = w3t1.astype(NPBF)
    # SEL (96, 32): st_all row 8*(3q+P) + 4*aa + l -> atom col 8l+2q+aa,
    # value = pair scale
    sel = np.zeros((96, 32), np.float32)
    for q in range(4):
        for P in range(3):
            for aa in range(2):
                for l in range(4):
                    sel[8 * (3 * q + P) + 4 * aa + l,
                        8 * l + 2 * q + aa] = PAIR_SC[P]
    aux["sel"] = sel
    # mean/std tables (8, 72): row t*4+q, col 24*c+f -> slot q*24+f, coord 1+c
    mA = np.zeros((8, 72), np.float32)
    mB = np.zeros((8, 72), np.float32)
    for t in range(2):
        for q in range(4):
            for c in range(3):
                sl = slice(24 * c, 24 * c + 24)
                mA[t * 4 + q, sl] = 1.0 / stddev[t, q * 24:q * 24 + 24, 1 + c]
                mB[t * 4 + q, sl] = (mean[t, q * 24:q * 24 + 24, 1 + c]
                                     / stddev[t, q * 24:q * 24 + 24, 1 + c])
    aux["mstabA"] = mA
    aux["mstabB"] = mB
    aux["qvec"] = (np.arange(128, dtype=np.float32) % 4).reshape(128, 1)
    # selection matrices for env_bm construction: S_k1[r, j] = 1 iff
    # r == k1c*j + k1, replicated at the 32-aligned row bases
    def srep(R, k1c, nbases):
        nch = R // k1c
        out = np.zeros((k1c, 128, nch), np.float32)
        for k1 in range(k1c):
            s = np.zeros((R, nch), np.float32)
            for j in range(nch):
                s[k1c * j + k1, j] = 1.0
            for b in range(nbases):
                out[k1, (128 // nbases) * b:(128 // nbases) * b + R, :] = s
        return out
    aux["s0rep"] = srep(32, 4, 4).astype(NPBF)
    aux["s1rep"] = srep(32, 2, 4).astype(NPBF)
    aux["s2rep"] = srep(64, 2, 2).astype(NPBF)
    return aux


# ---------------------------------------------------------------- program

def build_program():
    nc = bacc.Bacc("TRN2", target_bir_lowering=False, debug=False,
                   enable_asserts=True, num_devices=NCORES)

    # DRAM IO (per-core values supplied via in_maps)
    coordT = nc.dram_tensor("coordT", [NALL, 3], F32, kind="ExternalInput").ap()
    atypeF = nc.dram_tensor("atypeF", [NALL, 1], F32, kind="ExternalInput").ap()
    nlist24 = nc.dram_tensor("nlist24", [128, 24], I32, kind="ExternalInput").ap()
    ownid = nc.dram_tensor("ownid", [128, 1], I32, kind="ExternalInput").ap()
    qvecD = nc.dram_tensor("qvec", [128, 1], F32, kind="ExternalInput").ap()
    mstabAD = nc.dram_tensor("mstabA", [8, 72], F32, kind="ExternalInput").ap()
    mstabBD = nc.dram_tensor("mstabB", [8, 72], F32, kind="ExternalInput").ap()
    w1repD = nc.dram_tensor("w1rep", [3, 128, 128], BF16, kind="ExternalInput").ap()
    b1tD = nc.dram_tensor("b1t", [3, 128, 1], F32, kind="ExternalInput").ap()
    w2pkD = nc.dram_tensor("w2pk", [3, 128, 128], BF16, kind="ExternalInput").ap()
    b2spD = nc.dram_tensor("b2sp", [3, 128, 1], F32, kind="ExternalInput").ap()
    w3catD = nc.dram_tensor("w3cat", [3, 128, 144], BF16, kind="ExternalInput").ap()
    w3t1D = nc.dram_tensor("w3t1", [3, 128, 144], BF16, kind="ExternalInput").ap()
    selD = nc.dram_tensor("sel", [96, 32], F32, kind="ExternalInput").ap()
    s0repD = nc.dram_tensor("s0rep", [4, 128, 8], BF16, kind="ExternalInput").ap()
    s1repD = nc.dram_tensor("s1rep", [2, 128, 16], BF16, kind="ExternalInput").ap()
    s2repD = nc.dram_tensor("s2rep", [2, 128, 32], BF16, kind="ExternalInput").ap()
    outD = nc.dram_tensor("out", [A_CORE, NG], F32, kind="ExternalOutput").ap()

    TANH = mybir.ActivationFunctionType.Tanh
    SQRT = mybir.ActivationFunctionType.Sqrt
    COPYF = mybir.ActivationFunctionType.Copy
    MUL = mybir.AluOpType.mult
    SUB = mybir.AluOpType.subtract
    ADD = mybir.AluOpType.add

    with tile.TileContext(nc) as tc, \
         tc.tile_pool(name="wpool", bufs=1) as wpool, \
         tc.tile_pool(name="s1", bufs=1) as s1, \
         tc.tile_pool(name="sbT1", bufs=2) as sbT1, \
         tc.tile_pool(name="sbT2", bufs=4) as sbT2, \
         tc.tile_pool(name="sbTT", bufs=4) as sbTT:

        # ---- persistent weights in SBUF
        def wtile(ap_dram, shape, dtype, tag):
            t = wpool.tile(shape, dtype, tag=tag)
            nc.sync.dma_start(t[:], ap_dram)
            return t

        w1sb = [wtile(w1repD[p], [128, 128], BF16, f"w1_{p}") for p in range(3)]
        b1sb = [wtile(b1tD[p], [128, 1], F32, f"b1_{p}") for p in range(3)]
        w2sb = [wtile(w2pkD[p], [128, 128], BF16, f"w2_{p}") for p in range(3)]
        b2sb = [wtile(b2spD[p], [128, 1], F32, f"b2_{p}") for p in range(3)]
        w3sb = [wtile(w3catD[p], [128, 144], BF16, f"w3_{p}") for p in range(3)]
        w3t1sb = [wtile(w3t1D[p], [128, 144], BF16, f"w3t1_{p}")
                  for p in range(3)]
        selsb = wtile(selD, [96, 32], F32, "sel")
        s0rep = [wtile(s0repD[k], [128, 8], BF16, f"s0_{k}") for k in range(4)]
        s1rep = [wtile(s1repD[k], [128, 16], BF16, f"s1_{k}") for k in range(2)]
        s2rep = [wtile(s2repD[k], [128, 32], BF16, f"s2_{k}") for k in range(2)]
        st_all = wpool.tile([96, 144], F32, tag="stall")

        # ---- stage 1: env construction
        with tc.tile_pool(name="ps1", bufs=2, space="PSUM") as ps1, \
             tc.tile_pool(name="psbm", bufs=2, space="PSUM") as psbm:
            nl = s1.tile([128, 24], I32, tag="nl")
            nc.sync.dma_start(nl[:], nlist24)
            oid = s1.tile([128, 1], I32, tag="oid")
            nc.sync.dma_start(oid[:], ownid)
            qv = s1.tile([128, 1], F32, tag="qv")
            nc.sync.dma_start(qv[:], qvecD)

            nbr = s1.tile([128, 72], F32, tag="nbr")
            for f in range(24):
                nc.gpsimd.indirect_dma_start(
                    out=nbr[:, 3 * f:3 * f + 3], out_offset=None, in_=coordT,
                    in_offset=bass.IndirectOffsetOnAxis(ap=nl[:, f:f + 1], axis=0))
            own = s1.tile([128, 3], F32, tag="own")
            nc.gpsimd.indirect_dma_start(
                out=own[:], out_offset=None, in_=coordT,
                in_offset=bass.IndirectOffsetOnAxis(ap=oid[:, 0:1], axis=0))
            tvec = s1.tile([128, 1], F32, tag="tvec")
            nc.gpsimd.indirect_dma_start(
                out=tvec[:], out_offset=None, in_=atypeF,
                in_offset=bass.IndirectOffsetOnAxis(ap=oid[:, 0:1], axis=0))
            # trow = 4*type + q  (as int32 for the gather)
            trowf = s1.tile([128, 1], F32, tag="trowf")
            nc.vector.tensor_scalar(out=trowf[:], in0=tvec[:], scalar1=4.0,
                                    scalar2=None, op0=MUL)
            nc.vector.tensor_add(trowf[:], trowf[:], qv[:])
            trow = s1.tile([128, 1], I32, tag="trow")
            nc.vector.tensor_copy(trow[:], trowf[:])
            Aexp = s1.tile([128, 72], F32, tag="Aexp")
            nc.gpsimd.indirect_dma_start(
                out=Aexp[:], out_offset=None, in_=mstabAD,
                in_offset=bass.IndirectOffsetOnAxis(ap=trow[:, 0:1], axis=0))
            Bexp = s1.tile([128, 72], F32, tag="Bexp")
            nc.gpsimd.indirect_dma_start(
                out=Bexp[:], out_offset=None, in_=mstabBD,
                in_offset=bass.IndirectOffsetOnAxis(ap=trow[:, 0:1], axis=0))

            d = [s1.tile([128, 24], F32, tag=f"d{c}", name=f"d{c}")
                 for c in range(3)]
            nbr3 = nbr[:].rearrange("p (f c) -> p f c", c=3)
            for c in range(3):
                nc.vector.tensor_tensor(
                    out=d[c][:], in0=nbr3[:, :, c],
                    in1=own[:, c:c + 1].to_broadcast([128, 24]), op=SUB)
            l2 = s1.tile([128, 24], F32, tag="l2")
            tmp = s1.tile([128, 24], F32, tag="tmp")
            nc.vector.tensor_tensor(out=l2[:], in0=d[0][:], in1=d[0][:], op=MUL)
            nc.vector.tensor_tensor(out=tmp[:], in0=d[1][:], in1=d[1][:], op=MUL)
            nc.vector.tensor_add(l2[:], l2[:], tmp[:])
            nc.vector.tensor_tensor(out=tmp[:], in0=d[2][:], in1=d[2][:], op=MUL)
            nc.vector.tensor_add(l2[:], l2[:], tmp[:])
            ll = s1.tile([128, 24], F32, tag="ll")
            nc.scalar.activation(ll[:], l2[:], SQRT)
            rinv2 = s1.tile([128, 24], F32, tag="rinv2")
            nc.vector.reciprocal(rinv2[:], l2[:])
            # smooth weight: uu=(l-rmin)/(rmax-rmin) clamped to [0,1],
            # vv = uu^3(-6uu^2+15uu-10)+1
            uu = s1.tile([128, 24], F32, tag="uu")
            sc = 1.0 / (RCUT - RCUT_SMTH)
            nc.scalar.activation(uu[:], ll[:], COPYF, bias=-RCUT_SMTH * sc, scale=sc)
            nc.vector.tensor_scalar(out=uu[:], in0=uu[:], scalar1=0.0, scalar2=1.0,
                                    op0=mybir.AluOpType.max, op1=mybir.AluOpType.min)
            poly = s1.tile([128, 24], F32, tag="poly")
            nc.vector.tensor_scalar(out=poly[:], in0=uu[:], scalar1=-6.0,
                                    scalar2=15.0, op0=MUL, op1=ADD)
            nc.vector.tensor_tensor(out=poly[:], in0=poly[:], in1=uu[:], op=MUL)
            nc.vector.tensor_scalar(out=poly[:], in0=poly[:], scalar1=10.0,
                                    scalar2=None, op0=SUB)
            u2 = s1.tile([128, 24], F32, tag="u2")
            nc.vector.tensor_tensor(out=u2[:], in0=uu[:], in1=uu[:], op=MUL)
            nc.vector.tensor_tensor(out=u2[:], in0=u2[:], in1=uu[:], op=MUL)
            nc.vector.tensor_tensor(out=poly[:], in0=poly[:], in1=u2[:], op=MUL)
            nc.vector.tensor_scalar(out=poly[:], in0=poly[:], scalar1=1.0,
                                    scalar2=None, op0=ADD)
            # s2 = sw / l^2 ; rr_c = (d_c * s2) * A_c - B_c
            s2 = s1.tile([128, 24], F32, tag="s2")
            nc.vector.tensor_tensor(out=s2[:], in0=poly[:], in1=rinv2[:], op=MUL)
            rrf = s1.tile([3, 3072], BF16, tag="rrf")
            rrt = s1.tile([128, 24], F32, tag="rrt")
            for c in range(3):
                rr = s1.tile([128, 24], BF16, tag="rr")
                nc.vector.tensor_tensor(out=rrt[:], in0=d[c][:], in1=s2[:], op=MUL)
                nc.vector.tensor_tensor(out=rrt[:], in0=rrt[:],
                                        in1=Aexp[:, 24 * c:24 * c + 24], op=MUL)
                nc.vector.tensor_tensor(out=rr[:], in0=rrt[:],
                                        in1=Bexp[:, 24 * c:24 * c + 24], op=SUB)
                nc.sync.dma_start(rrf[c:c + 1, :], rr[:])

            # pair matmuls -> envA (128 x 768) [4 atoms x (32 x 96) = env00|env01],
            #                 envB (128 x 1024) [2 atoms x (64 x 64) = env11]
            envA = s1.tile([128, 768], BF16, tag="envA")
            envB = s1.tile([128, 1024], BF16, tag="envB")
            for blk in range(8):          # 4-atom blocks
                pa = ps1.tile([128, 96], F32, tag="pa")
                for j in range(4):
                    a = 4 * blk + j
                    nc.tensor.matmul(
                        out=pa[32 * j:32 * j + 32, :],
                        lhsT=rrf[:, 96 * a:96 * a + 32],
                        rhs=rrf[:, 96 * a:96 * a + 96], start=True, stop=True,
                        tile_position=(0, 32 * j))
                nc.vector.tensor_copy(envA[:, 96 * blk:96 * blk + 96], pa[:])
            for blk in range(16):         # 2-atom blocks
                pb = ps1.tile([128, 64], F32, tag="pb")
                for j in range(2):
                    a = 2 * blk + j
                    nc.tensor.matmul(
                        out=pb[64 * j:64 * j + 64, :],
                        lhsT=rrf[:, 96 * a + 32:96 * a + 96],
                        rhs=rrf[:, 96 * a + 32:96 * a + 96], start=True,
                        stop=True, tile_position=(0, 64 * j))
                nc.vector.tensor_copy(envB[:, 64 * blk:64 * blk + 64], pb[:])

            # ---- flatten into the MLP batch order (plain row-major within
            # each (atom, pair) segment).
            # one DMA per (q, l, segment) moves BOTH atom-halves aa=0,1:
            # aa-paired segments are contiguous in the new env_mlp layout, and
            # the src rows linearize as (aa, j, k) in exactly dst order.
            env_mlp = s1.tile([128, 14336], BF16, tag="env_mlp")
            for q in range(4):
                for l in range(4):
                    row = 32 * q + l
                    rA = 64 * (q % 2)
                    cA = 96 * (2 * l + q // 2)
                    srcs = (
                        (0, 2048, envA[rA:rA + 64, cA:cA + 32]),
                        (2048, 4096, envA[rA:rA + 64, cA + 32:cA + 96]),
                        (6144, 8192, envB[:, 64 * (4 * l + q):
                                          64 * (4 * l + q) + 64]),
                    )
                    for off, seglen, src in srcs:
                        nc.sync.dma_start(
                            env_mlp[row:row + 1, off:off + seglen], src)

            # ---- env_bm per lane (128 x 3584), contiguous window sections:
            # window w occupies cols [WBASE[w], WBASE[w]+8*NCHW[w]); section r
            # holds the chunks whose acc row is r, at their window position.
            # Built on PE: chunk-column block = tile^T @ S_k1 (selection),
            # with the normal env tile as lhsT (psum rows C*k1 via col tiling).
            env_bm = []
            for l in range(4):
                eb = s1.tile([128, 3584], BF16, tag=f"env_bm{l}",
                             name=f"env_bm{l}")
                nc.vector.memset(eb[:], 0.0)
                env_bm.append(eb)
            for l in range(4):
                for ap_ in range(8):        # atom within lane (= 2q+aa)
                    a = 8 * l + ap_
                    for P in range(3):
                        q_, aa = ap_ // 2, ap_ % 2
                        w = 3 * q_ + P
                        nch = SEG_NCH[P]
                        r = 4 * aa + l
                        col0 = (WBASE[w] + r * NCHW[w]
                                + aa * SEG_NCH[P])
                        if P == 0:
                            tile_ = envA[32 * (a % 4):32 * (a % 4) + 32,
                                         96 * (a // 4):96 * (a // 4) + 32]
                            k1c, C, rb = 4, 32, 32 * (a % 4)
                            sreps = s0rep
                        elif P == 1:
                            tile_ = envA[32 * (a % 4):32 * (a % 4) + 32,
                                         96 * (a // 4) + 32:96 * (a // 4) + 96]
                            k1c, C, rb = 2, 64, 32 * (a % 4)
                            sreps = s1rep
                        else:
                            tile_ = envB[64 * (a % 2):64 * (a % 2) + 64,
                                         64 * (a // 2):64 * (a // 2) + 64]
                            k1c, C, rb = 2, 64, 64 * (a % 2)
                            sreps = s2rep
                        R = tile_.shape[0]
                        pbm = psbm.tile([128, nch], F32, tag="pbm", name="pbm")
                        for kk in range(k1c):
                            nc.tensor.matmul(
                                out=pbm[C * kk:C * kk + C, :],
                                lhsT=tile_,
                                rhs=sreps[kk][rb:rb + R, :],
                                start=True, stop=True,
                                tile_position=(rb, C * kk))
                        nc.vector.tensor_copy(env_bm[l][:, col0:col0 + nch],
                                              pbm[:])

        # ---- stage 2: MLP + contraction over 112 supertiles
        with tc.tile_pool(name="psL1", bufs=2, space="PSUM") as psL1, \
             tc.tile_pool(name="psL2", bufs=1, space="PSUM") as psL2, \
             tc.tile_pool(name="psP3", bufs=2, space="PSUM") as psP3, \
             tc.tile_pool(name="psacc", bufs=1, space="PSUM") as psacc:

            def emit_L1(s):
                q, si = s // 28, s % 28
                P = PAIR_OF_POS[s % 28]
                p1 = psL1.tile([128, 512], F32, tag="p1")
                nc.tensor.matmul(out=p1[:],
                                 lhsT=w1sb[P][32 * q:32 * q + 4, :],
                                 rhs=env_mlp[32 * q:32 * q + 4,
                                             512 * si:512 * si + 512],
                                 start=True, stop=True,
                                 tile_position=(32 * q, 0))
                return p1

            acc_tile = None
            win_open = -1
            p1_next = emit_L1(0)
            for s in range(NSUPER):
                q, si = s // 28, s % 28
                P = PAIR_OF_POS[s % 28]
                p1 = p1_next
                t1 = sbT1.tile([128, 512], BF16, tag="t1")
                nc.scalar.activation(t1[:], p1[:], TANH, bias=b1sb[P][:, 0:1])
                # software pipelining: issue next supertile's L1 now so PE
                # fills its stalls and t1(s+1) input is ready early
                if s + 1 < NSUPER:
                    p1_next = emit_L1(s + 1)
                t2s = []
                for t in range(2):
                    p2 = psL2.tile([128, 512], F32, tag="p2")
                    nc.tensor.matmul(out=p2[:],
                                     lhsT=w2sb[P][64 * t:64 * t + 64, :],
                                     rhs=t1[64 * t:64 * t + 64, :],
                                     start=True, stop=True,
                                     tile_position=(64 * t, 0))
                    t2 = sbT2.tile([128, 512], BF16, tag="t2")
                    nc.scalar.activation(t2[:], p2[:], TANH,
                                         bias=b2sb[P][:, 0:1])
                    t2s.append(t2)
                def emit_L3(l):
                    # p3 = W3^T tanh(p2) + W3sum^T h1 (+b3) via two
                    # PSUM-accumulating matmuls; the second also supplies the
                    # h2^T = tanh^T + [h1|h1]^T passthrough in cols 96:144.
                    t, par = l // 2, l % 2
                    rb = 64 * par
                    rt = 32 * l
                    p3 = psP3.tile([128, 1024], F32, tag="p3")
                    for qq in range(4):
                        nc.tensor.matmul(
                            out=p3[:, 256 * qq:256 * qq + 144],
                            lhsT=t2s[t][rb:rb + 64, 128 * qq:128 * qq + 128],
                            rhs=w3sb[P][rb:rb + 64, :], start=True, stop=False)
                        nc.tensor.matmul(
                            out=p3[:, 256 * qq:256 * qq + 144],
                            lhsT=t1[rt:rt + 24, 128 * qq:128 * qq + 128],
                            rhs=w3t1sb[P][rt:rt + 24, :],
                            start=False, stop=True,
                            tile_position=(rt, 0))
                    return p3

                p3_next = emit_L3(0)
                for l in range(4):
                    p3 = p3_next
                    tt = sbTT.tile([128, 576], BF16, tag="tt")
                    p3r = p3[:].rearrange("p (k x) -> p k x", k=4)
                    ttr = tt[:].rearrange("p (k x) -> p k x", k=4)
                    nc.scalar.activation(ttr[:, :, 0:96], p3r[:, :, 0:96], TANH)
                    nc.vector.tensor_copy(ttr[:, :, 96:144], p3r[:, :, 96:144])
                    # issue next lane's L3 before this lane's contraction so
                    # PE overlaps the tt activation
                    if l + 1 < 4:
                        p3_next = emit_L3(l + 1)
                    for qq in range(4):
                        cc = 4 * s + qq
                        ccr = cc % 112
                        Pc = _pair_of_ccr(ccr)
                        w = 3 * (cc // 112) + Pc
                        if w != win_open:
                            # flush previous window via SBUF staging
                            if acc_tile is not None:
                                stg = sbTT.tile([8, 144], F32, tag="stg",
                                                name="stg")
                                nc.vector.tensor_copy(stg[:], acc_tile[:])
                                nc.sync.dma_start(
                                    st_all[8 * win_open:8 * win_open + 8, :],
                                    stg[:])
                            acc_tile = psacc.tile([8, 144], F32, tag="acc")
                            win_open = w
                            first = True
                        else:
                            first = False
                        last = (ccr == SEG_CEND[Pc] - 1) and (l == 3)
                        # lhsT: 8 one-hot cols = window sections at this
                        # chunk's position (stride NCHW[w])
                        pos = cc - W0[w]
                        lhs = env_bm[l][:, WBASE[w]:WBASE[w] + 8 * NCHW[w]] \
                            .rearrange("p (r j) -> p j r", r=8)[:, pos, :]
                        nc.tensor.matmul(
                            out=acc_tile[:], lhsT=lhs,
                            rhs=tt[:, 144 * qq:144 * qq + 144],
                            start=first, stop=last, skip_group_check=True)
            # flush last window
            stg = sbTT.tile([8, 144], F32, tag="stg", name="stg")
            nc.vector.tensor_copy(stg[:], acc_tile[:])
            nc.sync.dma_start(st_all[8 * win_open:8 * win_open + 8, :], stg[:])

            # ---- stage 3: combine
            res_ps = psL1.tile([32, 144], F32, tag="p1")
            nc.tensor.matmul(out=res_ps[:], lhsT=selsb[:], rhs=st_all[:],
                             start=True, stop=True)
            res_cp = wpool.tile([32, 144], F32, tag="rescp")
            nc.vector.tensor_copy(res_cp[:], res_ps[:])
            res_sb = wpool.tile([32, 96], F32, tag="res")
            nc.vector.tensor_tensor(
                out=res_sb[:].rearrange("p (r f) -> p r f", r=2),
                in0=res_cp[:, 0:96].rearrange("p (r f) -> p r f", r=2),
                in1=res_cp[:, 96:144].rearrange("p (r f) -> p r f", r=1)
                    .to_broadcast([32, 2, 48]),
                op=ADD)
            nc.sync.dma_start(outD, res_sb[:])

    nc.compile()
    return nc


_CACHE = {}


def _get_program():
    if "nc" not in _CACHE:
        _CACHE["nc"] = build_program()
    return _CACHE["nc"]


def make_in_maps(nlist, extended_coord, extended_atype, mean, stddev,
                 W1, b1, W2, b2, W3, b3):
    nlist = np.asarray(nlist)
    aux = _build_static_aux(np.asarray(W1, np.float32), np.asarray(b1, np.float32),
                            np.asarray(W2, np.float32), np.asarray(b2, np.float32),
                            np.asarray(W3, np.float32), np.asarray(b3, np.float32),
                            np.asarray(mean, np.float32),
                            np.asarray(stddev, np.float32))
    coordT = np.asarray(extended_coord, np.float32).reshape(NALL, 3)
    atypeF = np.asarray(extended_atype).astype(np.float32).reshape(NALL, 1)

    in_maps = []
    for c in range(NCORES):
        m = {
            "coordT": coordT,
            "atypeF": atypeF,
            "nlist24": nlist[0, 32 * c:32 * c + 32, :].astype(np.int32)
                       .reshape(128, 24),
            "ownid": (32 * c + np.arange(128) // 4).astype(np.int32)
                     .reshape(128, 1),
            "qvec": aux["qvec"],
            "mstabA": aux["mstabA"], "mstabB": aux["mstabB"],
            "sel": aux["sel"],
            "s0rep": aux["s0rep"], "s1rep": aux["s1rep"],
            "s2rep": aux["s2rep"],
            "w1rep": aux["w1rep"], "b1t": aux["b1t"],
            "w2pk": aux["w2pk"], "b2sp": aux["b2sp"],
            "w3cat": aux["w3cat"], "w3t1": aux["w3t1"],
        }
        in_maps.append(m)
    return in_maps


def kernel_run(trace=False, **inputs):
    in_maps = make_in_maps(**inputs)
    nc = _get_program()
    res = run_bass_kernel_spmd(nc, in_maps, core_ids=list(range(NCORES)),
                               trace=trace)
    out = np.concatenate([res.results[c]["out"] for c in range(NCORES)], axis=0)
    return out.reshape(1, NLOC, NG).astype(np.float32), res


def kernel(nlist, extended_coord, extended_atype, mean, stddev,
           W1, b1, W2, b2, W3, b3):
    out, _ = kernel_run(
        nlist=nlist, extended_coord=extended_coord,
        extended_atype=extended_atype, mean=mean, stddev=stddev,
        W1=W1, b1=b1, W2=W2, b2=b2, W3=W3, b3=b3)
    return out
